# revision 48
# baseline (speedup 1.0000x reference)
"""Trainium2 Bass kernel for nn_CMP_3367254360436 (gnn_message_passing).

Reference computation: bidirectional signed scatter-add pooling over 8192
edges on 2048 nodes of [16,32,32] fp32 feature maps, concat [feats, pooled_pos,
pooled_neg] (48 ch), then three 3x3 SAME convs (48->32->32->16) with leaky
ReLU (0.1).

Device decomposition (per NeuronCore, 256 nodes/core in 64 quads of 4 nodes):
  1. Pooling: one dma_gather per quad pulls (contribution, channel) rows
     (idx = src_node*16 + ch) in bf16 from the full feats array into
     [128 rows, G, 1024] SBUF; compile-time 0/1 selection matrices S then
     accumulate rows into pooled (node, sign, ch) slots via bf16 matmuls
     accumulated in fp32 PSUM.
  2. Convs use a GAP layout: each 32x32 image is stored with row stride 33
     (one zero "gap" column after each pixel row) inside 34-wide zero
     guards, so every 3x3 tap is a pure free-dim offset with correct SAME
     padding and ALL taps of a layer accumulate into a single PSUM group.
     Each layer runs as 4 PSUM quarters of 264 columns (8 gap-rows, one
     2KB PSUM bank, >=256 keeps fp32r/bf16 matmul at 1 column/cycle).
     Weights are block-diagonal over the 4 nodes of the quad.
     conv1's feats operand (64 partitions) is loaded twice - partitions
     64..127 hold the +1-shifted copy - so kx=0/kx=1 tap pairs merge into
     single K=128 matmuls (6 instead of 9 matmuls per ky sweep).
     ACT evacuates each quarter with bias + Prelu(0.1) into the next
     layer's gap tile (bf16).
  3. conv3 output [64 = 4n x 16ch, 1024] fp32 is DMA'd back to HBM.

All SBUF data is bf16 (halves DMA/SBUF/DVE cost; matmul throughput is the
same 1 column/cycle; PSUM accumulation stays fp32). The Bass program is
identical on all 8 cores (SPMD); all per-core variation (node assignment,
S matrices, gather indices) is carried in the input data.
"""

import numpy as np

LAST_EXEC_TIME_NS = None
V, C, H, W = 2048, 16, 32, 32
NCORES = 8
NPQ = 4                      # nodes per quad
QPC = V // NCORES // NPQ     # quads per core = 64
GL = 34                      # left guard (covers delta = -34)
RS = 33                      # gap-layout row stride (32 px + 1 zero gap)
GWT = GL + RS * 32 + 1 + 34  # tile width: 34 + 1056 + 34 = 1124
QW = 8 * RS                  # psum quarter width = 264 (8 image rows)
QW3 = QW + RS                # conv3 extended quarter (ky-pair shift halo)
GWT3 = GL + 3 * QW + 34 + QW3  # H2 width: conv3 singles read to col 1157


# ---------------------------------------------------------------- host prep

def _host_prep(feats, edges, W1, b1, W2, b2, W3, b3):
    edges = np.asarray(edges).reshape(-1, 3)
    src, sign, dst = edges[:, 0], edges[:, 1], edges[:, 2]
    feats = np.ascontiguousarray(np.asarray(feats), dtype=np.float32)
    import ml_dtypes
    bf16 = ml_dtypes.bfloat16

    pos = [[] for _ in range(V)]
    neg = [[] for _ in range(V)]
    for s, sg, d in zip(src, sign, dst):
        buck = pos if sg > 0 else neg
        buck[int(d)].append(int(s))
        buck[int(s)].append(int(d))

    # Pooling is split per slot into D[s] full "planes" (the first D[s]
    # contributors of every (node,sign) group; elementwise-summed on the
    # Pool engine) plus ragged leftovers (scatter-added via S-matmuls on
    # PE). Quads group nodes of similar min(pos,neg) degree and slots group
    # quads of similar quad-min so D[s] (shared by all cores, SPMD) stays
    # close to each group's actual degree and leftovers stay small.
    mn = np.array([min(len(pos[v]), len(neg[v])) for v in range(V)])
    mx = np.array([max(len(pos[v]), len(neg[v])) for v in range(V)])
    order = np.argsort(-(mn * 64 + mx), kind="stable")
    nquads = NCORES * QPC
    quads = [[int(v) for v in order[4 * q:4 * q + 4]] for q in range(nquads)]
    qmin = np.array([min(mn[v] for v in q) for q in quads])
    # ascending: slot 0 gets the lowest-degree quads, so the software
    # pipeline fills behind the smallest gathers/reductions
    qorder = np.argsort(qmin, kind="stable")
    assign = np.array(qorder).reshape(QPC, NCORES)  # [slot, core] -> quad id

    DCAP = 8
    D = np.zeros(QPC, np.int64)
    for s in range(QPC):
        D[s] = min(DCAP, min(qmin[assign[s, c]] for c in range(NCORES)))

    slot_left = {}
    for s in range(QPC):
        for c in range(NCORES):
            rows = []
            for n_local, v in enumerate(quads[assign[s, c]]):
                for sgn, lst in ((0, pos[v]), (1, neg[v])):
                    for u in lst[int(D[s]):]:
                        for ch in range(16):
                            rows.append((u * 16 + ch, 32 * n_local + 16 * sgn + ch))
            slot_left[(c, s)] = rows
    GLq = np.zeros(QPC, dtype=np.int64)       # leftover S-matmul groups
    for s in range(QPC):
        GLq[s] = max((len(slot_left[(c, s)]) + 127) // 128
                     for c in range(NCORES))
    T = D + GLq                                # gather groups per slot
    Gtot = int(T.sum())
    toff = np.concatenate([[0], np.cumsum(T)]).astype(np.int64)
    sgoff = np.concatenate([[0], np.cumsum(GLq)]).astype(np.int64)
    SGtot = max(1, int(GLq.sum()))

    in_maps = []
    node_lists = []
    feats_bf = feats.reshape(V * 16, 1024).astype(bf16)
    for c in range(NCORES):
        idxs_pack = np.zeros((16, 8 * Gtot), np.int16)   # idx j -> [j%16, j//16]
        S_pack = np.zeros((SGtot * 128, 128), bf16)
        f_own = np.zeros((QPC * 64, 1024), bf16)
        nodes_c = []
        for s in range(QPC):
            base = int(toff[s])
            nodes = quads[assign[s, c]]
            nodes_c.append(nodes)
            # plane part: j = (base+gg)*128 + p, p = 32n+16sgn+ch
            for n_local, v in enumerate(nodes):
                for sgn, lst in ((0, pos[v]), (1, neg[v])):
                    for gg in range(int(D[s])):
                        u = lst[gg]
                        for ch in range(16):
                            jj = (base + gg) * 128 + 32 * n_local + 16 * sgn + ch
                            idxs_pack[jj % 16, jj // 16] = u * 16 + ch
            # leftover part: packed rows + S selection
            lbase = base + int(D[s])
            sbase = int(sgoff[s])
            for j, (srcidx, slot) in enumerate(slot_left[(c, s)]):
                jj = lbase * 128 + j
                idxs_pack[jj % 16, jj // 16] = srcidx
                S_pack[sbase * 128 + j, slot] = 1.0
            for n_local, v in enumerate(nodes):
                f_own[s * 64 + 16 * n_local: s * 64 + 16 * n_local + 16] = \
                    feats_bf[v * 16:v * 16 + 16]
        node_lists.append(nodes_c)
        in_maps.append({
            "feats_all": feats_bf,
            "feats_own": f_own,
            # replicated across the 8 Q7 cores (16 partitions each)
            "idxs_pack": np.tile(idxs_pack, (8, 1)),
            "s_pack": S_pack,
        })

    # weight packs, stored as [K, ntaps*M], block-diagonal over 4 nodes
    W1 = np.asarray(W1); W2 = np.asarray(W2); W3 = np.asarray(W3)
    wa1 = np.zeros((128, 9, 128), np.float32)   # conv1, pooled operand
    wb1p = np.zeros((128, 3, 128), np.float32)  # conv1 feats, kx=0/1 pairs
    wb1q = np.zeros((128, 1, 128), np.float32)  # conv1 feats, (0,2)+(1,2)
    wb1s = np.zeros((64, 1, 128), np.float32)   # conv1 feats, (2,2) single
    w2p = np.zeros((128, 9, 128), np.float32)
    # conv3: ky=0/ky=1 tap weights stacked in M (partials combined with a
    # 33-col shifted add at evac); ky=2 kx=0/1 pair stacked the same way in
    # a second PSUM tile, with the (2,2) single joining its low half.
    w3pr = np.zeros((128, 3, 128), np.float32)
    w3d = np.zeros((128, 128), np.float32)
    w3sg = np.zeros((128, 64), np.float32)
    for ky in range(3):
        for kx in range(3):
            t = 3 * ky + kx
            for n in range(4):
                wa1[32*n:32*n+32, t, 32*n:32*n+32] = W1[:, 16:48, ky, kx].T
                w2p[32*n:32*n+32, t, 32*n:32*n+32] = W2[:, :, ky, kx].T
        for n in range(4):
            wb1p[16*n:16*n+16, ky, 32*n:32*n+32] = W1[:, 0:16, ky, 0].T
            wb1p[64+16*n:64+16*n+16, ky, 32*n:32*n+32] = W1[:, 0:16, ky, 1].T
    for n in range(4):
        wb1q[16*n:16*n+16, 0, 32*n:32*n+32] = W1[:, 0:16, 0, 2].T
        wb1q[64+16*n:64+16*n+16, 0, 32*n:32*n+32] = W1[:, 0:16, 1, 2].T
        wb1s[16*n:16*n+16, 0, 32*n:32*n+32] = W1[:, 0:16, 2, 2].T
    for kx in range(3):
        for n in range(4):
            w3pr[32*n:32*n+32, kx, 16*n:16*n+16] = W3[:, :, 0, kx].T
            w3pr[32*n:32*n+32, kx, 64+16*n:64+16*n+16] = W3[:, :, 1, kx].T
    for n in range(4):
        w3d[32*n:32*n+32, 16*n:16*n+16] = W3[:, :, 2, 0].T
        w3d[32*n:32*n+32, 64+16*n:64+16*n+16] = W3[:, :, 2, 1].T
        w3sg[32*n:32*n+32, 16*n:16*n+16] = W3[:, :, 2, 2].T
    consts = {
        "wa1": wa1.reshape(128, 9 * 128).astype(bf16),
        "wb1p": wb1p.reshape(128, 3 * 128).astype(bf16),
        "wb1q": wb1q.reshape(128, 128).astype(bf16),
        "wb1s": wb1s.reshape(64, 128).astype(bf16),
        "w2p": w2p.reshape(128, 9 * 128).astype(bf16),
        "w3pr": w3pr.reshape(128, 3 * 128).astype(bf16),
        "w3d": w3d.astype(bf16),
        "w3sg": w3sg.astype(bf16),
        "b1t": np.tile(np.asarray(b1), 4).astype(np.float32).reshape(128, 1),
        "b2t": np.tile(np.asarray(b2), 4).astype(np.float32).reshape(128, 1),
        "b3t": np.tile(np.asarray(b3), 4).astype(np.float32).reshape(64, 1),
    }
    for m in in_maps:
        m.update({k: v.copy() for k, v in consts.items()})
    meta = dict(D=D, GLq=GLq, toff=toff, sgoff=sgoff, Gtot=Gtot, SGtot=SGtot)
    return in_maps, node_lists, meta


# ------------------------------------------------------------- bass program

def _build_program(meta, nslots=QPC, repeat=1):
    import concourse.mybir as mybir
    from concourse import bacc
    from concourse.tile import TileContext

    D, GLq, toff, sgoff, Gtot, SGtot = (
        meta["D"], meta["GLq"], meta["toff"], meta["sgoff"],
        meta["Gtot"], meta["SGtot"])

    f32 = mybir.dt.float32
    bf16 = mybir.dt.bfloat16
    nc = bacc.Bacc("TRN2", target_bir_lowering=False)

    feats_all = nc.dram_tensor("feats_all", [V * 16, 1024], bf16,
                               kind="ExternalInput")
    feats_own = nc.dram_tensor("feats_own", [QPC * 64, 1024], bf16,
                               kind="ExternalInput")
    idxs_pack = nc.dram_tensor("idxs_pack", [128, 8 * Gtot], mybir.dt.int16,
                               kind="ExternalInput")
    s_pack = nc.dram_tensor("s_pack", [SGtot * 128, 128], bf16,
                            kind="ExternalInput")
    wa1 = nc.dram_tensor("wa1", [128, 9 * 128], bf16, kind="ExternalInput")
    wb1p = nc.dram_tensor("wb1p", [128, 3 * 128], bf16, kind="ExternalInput")
    wb1q = nc.dram_tensor("wb1q", [128, 128], bf16, kind="ExternalInput")
    wb1s = nc.dram_tensor("wb1s", [64, 128], bf16, kind="ExternalInput")
    w2p = nc.dram_tensor("w2p", [128, 9 * 128], bf16, kind="ExternalInput")
    w3pr = nc.dram_tensor("w3pr", [128, 3 * 128], bf16, kind="ExternalInput")
    w3d = nc.dram_tensor("w3d", [128, 128], bf16, kind="ExternalInput")
    w3sg = nc.dram_tensor("w3sg", [128, 64], bf16, kind="ExternalInput")
    b1t = nc.dram_tensor("b1t", [128, 1], f32, kind="ExternalInput")
    b2t = nc.dram_tensor("b2t", [128, 1], f32, kind="ExternalInput")
    b3t = nc.dram_tensor("b3t", [64, 1], f32, kind="ExternalInput")
    out_own = nc.dram_tensor("out_own", [QPC * 64, 1024], f32,
                             kind="ExternalOutput")

    # HW probe (prior session): Lrelu ignores the alpha operand (table slope
    # 0.01); Prelu honors alpha and matches leaky(0.1) exactly.
    LRELU = mybir.ActivationFunctionType.Prelu
    Tmax = int((D + GLq).max())
    GLmax = max(1, int(GLq.max()))

    def delta(ky, kx):
        return RS * (ky - 1) + (kx - 1)

    def gaps_view(t, p0, p1, col0):
        # gap columns col0 + RS*r, one after each of the 32 pixel rows
        return t[p0:p1, col0:col0 + RS * 32].rearrange(
            "p (r c) -> p r c", c=RS)[:, :, 0:1]

    def pix_view(t, p0, p1, col0, nrows=32):
        # [p, nrows, 32] pixel view of gap-layout columns starting at col0
        return t[p0:p1, col0:col0 + RS * nrows].rearrange(
            "p (r c) -> p r c", c=RS)[:, :, 0:32]

    with TileContext(nc) as tc:
        with (
            tc.tile_pool(name="const", bufs=1) as constp,
            tc.tile_pool(name="gath", bufs=3) as gathp,
            tc.tile_pool(name="stile", bufs=3) as stilep,
            tc.tile_pool(name="xt", bufs=3) as xtp,
            tc.tile_pool(name="acc", bufs=2) as accp,
            tc.tile_pool(name="comb", bufs=3) as combp,
            tc.tile_pool(name="otile", bufs=3) as otp,
            tc.tile_pool(name="poolps", bufs=1, space="PSUM") as poolpsp,
            tc.tile_pool(name="convps", bufs=4, space="PSUM") as convpsp,
            tc.tile_pool(name="convps2", bufs=2, space="PSUM") as convps2p,
        ):
            # ---- resident constants; idx first: slot 0's gather depends
            # on it, while weights are not needed until the first conv
            idx_t = constp.tile([128, 8 * Gtot], mybir.dt.int16)
            nc.sync.dma_start(out=idx_t[:, :], in_=idxs_pack[:, :])
            wa1_t = constp.tile([128, 9 * 128], bf16)
            wb1p_t = constp.tile([128, 3 * 128], bf16)
            wb1q_t = constp.tile([128, 128], bf16)
            wb1s_t = constp.tile([64, 128], bf16)
            w2p_t = constp.tile([128, 9 * 128], bf16)
            w3pr_t = constp.tile([128, 3 * 128], bf16)
            w3d_t = constp.tile([128, 128], bf16)
            w3sg_t = constp.tile([128, 64], bf16)
            nc.sync.dma_start(out=wa1_t[:, :], in_=wa1[:, :])
            nc.sync.dma_start(out=wb1p_t[:, :], in_=wb1p[:, :])
            nc.sync.dma_start(out=wb1q_t[:, :], in_=wb1q[:, :])
            nc.sync.dma_start(out=wb1s_t[:, :], in_=wb1s[:, :])
            nc.sync.dma_start(out=w2p_t[:, :], in_=w2p[:, :])
            nc.sync.dma_start(out=w3pr_t[:, :], in_=w3pr[:, :])
            nc.sync.dma_start(out=w3d_t[:, :], in_=w3d[:, :])
            nc.sync.dma_start(out=w3sg_t[:, :], in_=w3sg[:, :])
            b1_t = constp.tile([128, 1], f32)
            b2_t = constp.tile([128, 1], f32)
            b3_t = constp.tile([64, 1], f32)
            nc.sync.dma_start(out=b1_t[:, :], in_=b1t[:, :])
            nc.sync.dma_start(out=b2_t[:, :], in_=b2t[:, :])
            nc.sync.dma_start(out=b3_t[:, :], in_=b3t[:, :])

            def conv_layer(taps, M, bias, out_tile, out_pix_col0, out_f32):
                """taps: list of (lhsT_ap_fn(t), rhs_tile, rhs_p1, dlt).
                One PSUM group per 264-wide quarter; ACT evacuates with
                bias + leaky into out_tile's pixel columns."""
                for q in range(4):
                    ps = convpsp.tile([128, QW], f32, tag="convps",
                                      name=f"ps_{q}")
                    nmm = len(taps)
                    for i, (lw, rt, rp1, dlt) in enumerate(taps):
                        a = GL + QW * q + dlt
                        nc.tensor.matmul(
                            ps[:M, :], lw, rt[0:rp1, a:a + QW],
                            start=(i == 0), stop=(i == nmm - 1),
                        )
                    in_v = ps[:M, :].rearrange("p (r c) -> p r c",
                                               c=RS)[:, :, 0:32]
                    if out_f32:
                        out_v = out_tile[:M, 256 * q:256 * (q + 1)].rearrange(
                            "p (r c) -> p r c", c=32)
                    else:
                        out_v = pix_view(out_tile, 0, M, GL + QW * q, nrows=8)
                    nc.scalar.activation(out_v, in_v, LRELU,
                                         bias=bias[:M, :], alpha=0.1)

            tiles = {}   # slot -> dict of live tiles (3-step stagger)

            def stage_pool(s):
                """Gather + leftover S-matmuls + plane reduce -> P, FF."""
                d = int(D[s])
                gl = int(GLq[s])
                base = int(toff[s])
                sbase = int(sgoff[s])
                t_s = d + gl
                gath = None
                pool_ps = None
                # ---- pooling gather: [128, d+gl, 1024] bf16
                #      planes [0,d) + leftover packed rows [d, d+gl)
                if t_s:
                    gath = gathp.tile([128, Tmax * 1024], bf16, tag="gath",
                                      name="gath")
                    nc.gpsimd.dma_gather(
                        out_ap=gath[:, :t_s * 1024].rearrange(
                            "p (gg f) -> p gg f", f=1024),
                        in_ap=feats_all[:, :],
                        idxs_ap=idx_t[:, base * 8:(base + t_s) * 8],
                        num_idxs=t_s * 128,
                        num_idxs_reg=t_s * 128,
                        elem_size=1024,
                        single_packet=False,
                    )
                if gl:
                    s_t = stilep.tile([128, GLmax * 128], bf16, tag="stile",
                                      name="s_mat")
                    nc.sync.dma_start(
                        out=s_t[:, :gl * 128].rearrange("p (gg m) -> p gg m",
                                                        m=128),
                        in_=s_pack[sbase * 128:(sbase + gl) * 128, :].rearrange(
                            "(gg p) m -> p gg m", p=128),
                    )
                    pool_ps = poolpsp.tile([128, 1024], f32, tag="poolps",
                                           name="pool_ps")
                    for w0 in (0, 512):
                        for gg in range(gl):
                            nc.tensor.matmul(
                                pool_ps[:, w0:w0 + 512],
                                s_t[:, gg * 128:(gg + 1) * 128],
                                gath[:, (d + gg) * 1024 + w0:
                                     (d + gg) * 1024 + w0 + 512],
                                start=(gg == 0), stop=(gg == gl - 1),
                            )
                # ---- plane reduction on the Pool engine (fp32 accumulator)
                if d >= 2:
                    acc = accp.tile([128, 1024], f32, tag="acc", name="acc")
                    nc.gpsimd.tensor_tensor(
                        out=acc[:, :], in0=gath[:, 0:1024],
                        in1=gath[:, 1024:2048], op=mybir.AluOpType.add)
                    for gg in range(2, d):
                        nc.gpsimd.tensor_tensor(
                            out=acc[:, :], in0=acc[:, :],
                            in1=gath[:, gg * 1024:(gg + 1) * 1024],
                            op=mybir.AluOpType.add)

                # ---- gap-layout input tiles
                P = xtp.tile([128, GWT], bf16, tag="P", name="P")
                nc.vector.memset(P[:, 0:GL], 0.0)
                nc.vector.memset(P[:, GL + RS * 32: GWT], 0.0)
                nc.vector.memset(gaps_view(P, 0, 128, GL + 32), 0.0)
                ppix = pix_view(P, 0, 128, GL)
                if d >= 2:
                    part = acc[:, :].rearrange("p (r c) -> p r c", c=32)
                elif d == 1:
                    part = gath[:, 0:1024].rearrange("p (r c) -> p r c", c=32)
                else:
                    part = None
                psv = (pool_ps[:, :].rearrange("p (r c) -> p r c", c=32)
                       if gl else None)
                if part is not None and psv is not None:
                    nc.vector.tensor_tensor(out=ppix, in0=part, in1=psv,
                                            op=mybir.AluOpType.add)
                elif part is not None:
                    nc.vector.tensor_copy(out=ppix, in_=part)
                elif psv is not None:
                    nc.vector.tensor_copy(out=ppix, in_=psv)
                else:
                    nc.vector.memset(ppix, 0.0)

                FF = xtp.tile([128, GWT], bf16, tag="FF", name="FF")
                nc.vector.memset(FF[0:64, 0:GL], 0.0)
                nc.vector.memset(FF[64:128, 0:GL - 1], 0.0)
                nc.vector.memset(FF[0:64, GL + RS * 32: GWT], 0.0)
                nc.vector.memset(FF[64:128, GL + RS * 32 - 1: GWT], 0.0)
                nc.vector.memset(gaps_view(FF, 0, 64, GL + 32), 0.0)
                nc.vector.memset(gaps_view(FF, 64, 128, GL + 31), 0.0)
                f_src = feats_own[s * 64:(s + 1) * 64, :].rearrange(
                    "p (r c) -> p r c", c=32)
                nc.sync.dma_start(out=pix_view(FF, 0, 64, GL), in_=f_src)
                nc.sync.dma_start(out=pix_view(FF, 64, 128, GL - 1),
                                  in_=f_src)

                # FS: feats (rows 0:64) + one-row (+33) shifted copy (rows
                # 64:128) so the (0,2)/(1,2) tap pair merges into one K=128
                # matmul.
                FS = xtp.tile([128, GWT], bf16, tag="FS", name="FS")
                nc.vector.memset(FS[0:64, 0:GL], 0.0)
                nc.vector.memset(FS[0:64, GL + RS * 32: GWT], 0.0)
                nc.vector.memset(gaps_view(FS, 0, 64, GL + 32), 0.0)
                nc.vector.memset(FS[64:128, 0:1], 0.0)
                nc.vector.memset(FS[64:128, GL + RS * 32 - 33: GWT], 0.0)
                nc.vector.memset(gaps_view(FS, 64, 128, GL - 1), 0.0)
                nc.sync.dma_start(out=pix_view(FS, 0, 64, GL), in_=f_src)
                nc.sync.dma_start(out=pix_view(FS, 64, 128, GL - 33),
                                  in_=f_src)
                tiles[s] = {"P": P, "FF": FF, "FS": FS}

            def stage_l1(s):
                # ---- conv1: pooled (9 taps) + feats (4 pairs + 1 single)
                P, FF, FS = tiles[s]["P"], tiles[s]["FF"], tiles[s]["FS"]
                H1 = xtp.tile([128, GWT], bf16, tag="H1", name="H1")
                nc.vector.memset(H1[:, 0:GL], 0.0)
                nc.vector.memset(H1[:, GL + RS * 32: GWT], 0.0)
                nc.vector.memset(gaps_view(H1, 0, 128, GL + 32), 0.0)
                # feats taps first: FF/FS are ready after a fast DMA while
                # P waits on the gather->reduce->combine chain
                taps1 = []
                for ky in range(3):
                    taps1.append((wb1p_t[:, ky * 128:(ky + 1) * 128], FF, 128,
                                  delta(ky, 0)))
                taps1.append((wb1q_t[:, 0:128], FS, 128, delta(0, 2)))
                taps1.append((wb1s_t[:, 0:128], FF, 64, delta(2, 2)))
                for ky in range(3):
                    for kx in range(3):
                        t = 3 * ky + kx
                        taps1.append((wa1_t[:, t * 128:(t + 1) * 128], P, 128,
                                      delta(ky, kx)))
                conv_layer(taps1, 128, b1_t, H1, GL, False)
                tiles[s]["H1"] = H1

            def stage_l2(s):
                H1 = tiles[s]["H1"]
                H2 = xtp.tile([128, GWT3], bf16, tag="H2", name="H2")
                nc.vector.memset(H2[:, 0:GL], 0.0)
                nc.vector.memset(H2[:, GL + RS * 32: GWT3], 0.0)
                nc.vector.memset(gaps_view(H2, 0, 128, GL + 32), 0.0)
                taps2 = [(w2p_t[:, t * 128:(t + 1) * 128], H1, 128,
                          delta(t // 3, t % 3)) for t in range(9)]
                conv_layer(taps2, 128, b2_t, H2, GL, False)
                tiles[s]["H2"] = H2

            def stage_l3(s):
                # conv3: ky=0/1 taps stacked in M (3 K=128,M=128 matmuls),
                # ky=2 taps as M=64 singles accumulating into the low half.
                # psum half B holds the ky=1 partial shifted by -33; a DVE
                # add re-aligns it before the ACT bias+leaky evac.
                H2 = tiles[s]["H2"]
                OT = otp.tile([64, 1024], f32, tag="OT", name="OT")
                for q in range(4):
                    ps = convpsp.tile([128, QW3], f32, tag="convps",
                                      name=f"ps3_{q}")
                    P0 = GL + QW * q
                    for kx in range(3):
                        a = P0 + delta(0, kx)
                        nc.tensor.matmul(
                            ps[:, :], w3pr_t[:, kx * 128:(kx + 1) * 128],
                            H2[:, a:a + QW3],
                            start=(kx == 0), stop=(kx == 2))
                    psd = convps2p.tile([128, QW3], f32, tag="convps2",
                                        name=f"psd_{q}")
                    nc.tensor.matmul(psd[:, :], w3d_t[:, :],
                                     H2[:, P0 + 32:P0 + 32 + QW3],
                                     start=True, stop=False)
                    nc.tensor.matmul(psd[:64, :], w3sg_t[:, :],
                                     H2[:, P0 + 34:P0 + 34 + QW3],
                                     start=False, stop=True)
                    # two PSUM operands in one TensorTensor are rejected by
                    # the BIR verifier: stage each PSUM term via SBUF adds
                    tmp = combp.tile([64, QW], f32, tag="tmp", name="tmp")
                    nc.vector.tensor_copy(out=tmp[:, :],
                                          in_=ps[64:128, RS:RS + QW])
                    nc.vector.tensor_tensor(
                        out=tmp[:, :], in0=ps[0:64, 0:QW],
                        in1=tmp[:, :], op=mybir.AluOpType.add)
                    nc.vector.tensor_tensor(
                        out=tmp[:, :], in0=psd[64:128, 1:1 + QW],
                        in1=tmp[:, :], op=mybir.AluOpType.add)
                    nc.vector.tensor_tensor(
                        out=tmp[:, :], in0=psd[0:64, 0:QW],
                        in1=tmp[:, :], op=mybir.AluOpType.add)
                    in_v = tmp[:, :].rearrange("p (r c) -> p r c",
                                               c=RS)[:, :, 0:32]
                    out_v = OT[:64, 256 * q:256 * (q + 1)].rearrange(
                        "p (r c) -> p r c", c=32)
                    nc.scalar.activation(out_v, in_v, LRELU,
                                         bias=b3_t[:64, :], alpha=0.1)
                nc.sync.dma_start(out=out_own[s * 64:(s + 1) * 64, :],
                                  in_=OT[:, :])
                del tiles[s]

            # 3-step software pipeline: every PE instruction's inputs were
            # produced a full step earlier, so layer-boundary evac latency
            # never stalls the Tensor engine. `repeat` re-runs the whole
            # computation (identical output) to amplify device time for
            # dispatch-overhead-free benchmarking.
            for _ in range(repeat):
                for s in range(nslots + 3):
                    if s >= 1 and s - 1 < nslots:
                        stage_l1(s - 1)
                    if s < nslots:
                        stage_pool(s)
                    if s >= 2 and s - 2 < nslots:
                        stage_l2(s - 2)
                    if s >= 3 and s - 3 < nslots:
                        stage_l3(s - 3)
    nc.finalize()
    return nc


# ------------------------------------------------------------- entry point

def kernel(feats, edges, W1, b1, W2, b2, W3, b3):
    import sys
    if "/opt/trn_rl_repo" not in sys.path:
        sys.path.insert(0, "/opt/trn_rl_repo")
    from concourse.bass_utils import run_bass_kernel_spmd

    in_maps, node_lists, meta = _host_prep(
        feats, edges, W1, b1, W2, b2, W3, b3)
    nc = _build_program(meta)
    res = run_bass_kernel_spmd(nc, in_maps, core_ids=list(range(NCORES)))
    global LAST_EXEC_TIME_NS
    LAST_EXEC_TIME_NS = res.exec_time_ns
    out = np.zeros((V, C, H, W), np.float32)
    for c in range(NCORES):
        oo = np.asarray(res.results[c]["out_own"]).reshape(QPC, 64, 1024)
        for s in range(QPC):
            for n_local, v in enumerate(node_lists[c][s]):
                out[v] = oo[s, 16 * n_local:16 * n_local + 16].reshape(
                    16, 32, 32)
    return out
